# revision 30
# baseline (speedup 1.0000x reference)
"""Trainium2 Bass kernel for the ALayer problem (v2: fused conv1 via row-tiles).

Math (per image):
  y   = sigmoid(fc_w2 @ relu(fc_w1 @ mean_hw(x)))          # [576] channel attn
  A   = sigmoid(conv3x3(relu(conv3x3(x, se_w1)), se_w2))   # [H,W] spatial attn
  out[o,l] = A[l] * sum_{c,t} (weight[o,c,t] * y[c*9+t]) * xpad[c, l+dt]

Strategy: data-parallel, 2 images per core (8 cores).  Channels live on
SBUF partitions: img0 on partitions 0-63, img1 on 64-127.  Per conv tap
the main conv and the SE conv1 run as one matmul per image on the two
64-row halves of the PE array (tile_position (0,0) / (64,0)), which the
hardware executes concurrently and which fuses conv1 for free:
  chain-i0 -> psum1: relu1_i0 @ cols 0-15,  main_i0 @ cols 64-127
  chain-i1 -> psum2: main_i1  @ cols 0-63,  relu1_i1 @ cols 64-79
conv2 is a full-K=128 block-diag matmul over the relu1 buffer producing
A_i0 replicated on partitions 64-127 and A_i1 on 0-63 — exactly aligned
with the main-conv psums, so the epilogue is two half-width DVE
multiplies fused with PSUM eviction.  The channel attention y is folded
into per-image effective weights (ACT per-partition scale).  All matmul
operands are bf16 (fp32 matmul is 4x slower on TRN2 PE).
"""

import numpy as np

try:
    import concourse.bass as bass
except ImportError:  # pragma: no cover
    import sys

    sys.path.insert(0, "/opt/trn_rl_repo")
    import concourse.bass as bass

import concourse.mybir as mybir
from concourse import bacc
from concourse.bass_utils import run_bass_kernel_spmd
from concourse.tile import TileContext

F32 = mybir.dt.float32
BF16 = mybir.dt.bfloat16
AF = mybir.ActivationFunctionType

B, C, H, W = 16, 64, 128, 128
N_CORES = 8
BPC = B // N_CORES  # images per core = 2
NT = H // 4  # 32 spatial tiles of 4 image rows (512 px) each

_CACHED = {}
TRACE = False


def _build_nc():
    nc = bacc.Bacc(None, target_bir_lowering=False, debug=False)
    x_ext = nc.declare_dram_parameter("x", [BPC, C, H, W], F32, isOutput=False)
    wch_ext = nc.declare_dram_parameter("wch", [128, 9, 128], F32, isOutput=False)
    se2_ext = nc.declare_dram_parameter("se2blk", [128, 9, 128], F32, isOutput=False)
    fc1_ext = nc.declare_dram_parameter("fc1t", [128, 4], F32, isOutput=False)
    fc2_ext = nc.declare_dram_parameter("fc2t", [4, 576], F32, isOutput=False)
    out_ext = nc.declare_dram_parameter("out", [BPC, C, H, W], F32, isOutput=True)

    xv = x_ext[:].rearrange("b c h w -> (b c) h w")  # [128, 128, 128]
    ov = out_ext[:].rearrange("b c h w -> (b c) h w")

    taps = [(di, dj) for di in range(3) for dj in range(3)]

    with TileContext(nc) as tc:
        with (
            tc.tile_pool(name="persist", bufs=1) as pp,
            tc.tile_pool(name="stage", bufs=3) as sp,
            tc.tile_pool(name="asb", bufs=2) as ap_pool,
            tc.tile_pool(name="io", bufs=3) as iop,
            tc.tile_pool(name="psA", bufs=1, space="PSUM") as psA,
            tc.tile_pool(name="psM1", bufs=3, space="PSUM") as psM1,
            tc.tile_pool(name="psM2", bufs=3, space="PSUM") as psM2,
            tc.tile_pool(name="psY", bufs=1, space="PSUM") as psY,
        ):
            # ---- persistent SBUF tiles
            xpad = pp.tile([128, H + 2, W + 2], BF16)  # padded x, both imgs
            r1pad = pp.tile([128, H + 2, W + 2], BF16)  # padded relu(conv1)
            wch_f = pp.tile([128, 9, 128], F32)
            wch_b = pp.tile([128, 9, 128], BF16)
            # per-tap chain lhsT as separate tiles so tile-0 matmuls only
            # wait on tap-0's scaling, not all nine
            weff = [
                pp.tile([128, 128], BF16, name=f"weff{t}", tag=f"weff{t}")
                for t in range(9)
            ]
            se2_f = pp.tile([128, 9, 128], F32)
            se2_b = pp.tile([128, 9, 128], BF16)
            fc1_f = pp.tile([128, 4], F32)
            fc1_b = pp.tile([128, 4], BF16)
            fc2_f = pp.tile([4, 576], F32)
            fc2_b = pp.tile([4, 576], BF16)
            sums = pp.tile([128, 8], F32)
            sumtot = pp.tile([128, 1], F32)
            mean_b = pp.tile([128, 1], BF16)
            y1sb = pp.tile([4, 2], BF16)
            y2sb = pp.tile([1, 1152], F32)  # img0: 0-576, img1: 576-1152
            y2mat = pp.tile([128, 9], F32)

            # Zero only xpad's padding border (casts overwrite the interior);
            # r1pad must be fully zeroed (conv2 contracts its unwritten
            # partitions against zero lhsT rows, and SBUF garbage may be NaN).
            # All on the otherwise-idle gpsimd engine, off the cast path.
            nc.gpsimd.memset(xpad[:, 0:1, :], 0.0)
            nc.gpsimd.memset(xpad[:, H + 1 : H + 2, :], 0.0)
            nc.gpsimd.memset(xpad[:, :, 0:1], 0.0)
            nc.gpsimd.memset(xpad[:, :, W + 1 : W + 2], 0.0)
            nc.gpsimd.memset(r1pad[:], 0.0)

            # ---- x load (f32) -> cast to padded bf16 (DVE), accumulate sums
            for j in range(8):
                st = sp.tile([128, 16, W], F32, tag="xstage")
                dma_eng = nc.sync if j % 2 == 0 else nc.scalar
                dma_eng.dma_start(out=st[:], in_=xv[:, 16 * j : 16 * j + 16, :])
                nc.vector.scalar_tensor_tensor(
                    out=xpad[:, 1 + 16 * j : 17 + 16 * j, 1 : W + 1],
                    in0=st[:],
                    scalar=0.0,
                    in1=st[:],
                    op0=mybir.AluOpType.add,
                    op1=mybir.AluOpType.bypass,
                    accum_out=sums[:, j : j + 1],
                )

            # ---- parameter loads + bf16 casts (ACT is idle during x-load)
            for ext, ft, bt in (
                (fc1_ext, fc1_f, fc1_b),
                (fc2_ext, fc2_f, fc2_b),
                (wch_ext, wch_f, wch_b),
            ):
                nc.sync.dma_start(out=ft[:], in_=ext[:])
                nc.scalar.activation(out=bt[:], in_=ft[:], func=AF.Copy)
            nc.sync.dma_start(out=se2_f[:], in_=se2_ext[:])
            # static (un-y-scaled) parts of the chain lhsT: the se1 columns.
            # DVE, issued after the x-casts, so the ACT FIFO stays clear for
            # the y-chain's relu/sigmoid ops.
            for t in range(9):
                nc.vector.tensor_copy(weff[t][0:64, 0:16], wch_b[0:64, t, 0:16])
                nc.vector.tensor_copy(
                    weff[t][64:128, 64:80], wch_b[64:128, t, 64:80]
                )
            nc.vector.reduce_sum(
                out=sumtot[:], in_=sums[:], axis=mybir.AxisListType.X
            )
            nc.vector.tensor_copy(mean_b[:], sumtot[:])  # 1/HW folded into fc1t

            # ---- channel-attention chain: y2 = sigmoid(fc2 @ relu(fc1 @ mean))
            for img in range(2):
                yp = psY.tile([4, 1], F32, tag="y")
                nc.tensor.matmul(
                    yp[:],
                    lhsT=fc1_b[64 * img : 64 * img + 64, :],
                    rhs=mean_b[64 * img : 64 * img + 64, :],
                    start=True,
                    stop=True,
                )
                nc.scalar.activation(
                    out=y1sb[:, img : img + 1], in_=yp[:], func=AF.Relu
                )
            for img in range(2):
                for hh in range(2):
                    yp = psY.tile([1, 288], F32, tag="y")
                    nc.tensor.matmul(
                        yp[:],
                        lhsT=y1sb[:, img : img + 1],
                        rhs=fc2_b[:, 288 * hh : 288 * hh + 288],
                        start=True,
                        stop=True,
                    )
                    nc.scalar.activation(
                        out=y2sb[:, 576 * img + 288 * hh : 576 * img + 288 * hh + 288],
                        in_=yp[:],
                        func=AF.Sigmoid,
                    )
            # scatter y2 [1, 1152] -> [128, 9] (partition-major (img, c), free t)
            nc.gpsimd.dma_start(
                out=y2mat[:, :],
                in_=y2sb[:, :].rearrange("p (c t) -> p c t", c=128, t=9),
            )
            # ---- y-scaled main-conv weights (the W columns of each chain)
            for t in range(9):
                nc.scalar.activation(
                    out=weff[t][0:64, 64:128],
                    in_=wch_b[0:64, t, 64:128],
                    func=AF.Copy,
                    scale=y2mat[0:64, t : t + 1],
                )
                nc.vector.tensor_scalar_mul(
                    weff[t][64:128, 0:64],
                    wch_b[64:128, t, 0:64],
                    y2mat[64:128, t : t + 1],
                )
            nc.scalar.activation(out=se2_b[:], in_=se2_f[:], func=AF.Copy)

            # ---- main loop: fused main+conv1 chains, then conv2/epilogue
            # lagged by one tile (conv2 of tile k needs relu1 rows from tile
            # k+1's eviction).
            lag = []  # (k, psum1, psum2)

            def epilogue(k, p1, p2):
                r0 = 4 * k
                ps_a = psA.tile([128, 4, W], F32, tag="A")
                for t, (di, dj) in enumerate(taps):
                    nc.tensor.matmul(
                        ps_a[:],
                        lhsT=se2_b[:, t, :],
                        rhs=r1pad[:, r0 + di : r0 + di + 4, dj : dj + W],
                        start=(t == 0),
                        stop=(t == 8),
                    )
                asb = ap_pool.tile([128, 4, W], F32, tag="Asb")
                nc.scalar.activation(out=asb[:], in_=ps_a[:], func=AF.Sigmoid)
                osb = iop.tile([128, 4, W], F32, tag="osb")
                nc.vector.tensor_mul(osb[64:128], p1[64:128], asb[64:128])
                nc.vector.tensor_mul(osb[0:64], p2[0:64], asb[0:64])
                # chain-i1's output (osb 0-63) is shard slot 0's channels in
                # dram, because kernel() swaps the two images in the shard.
                out_eng = nc.sync if k % 2 == 0 else nc.scalar
                out_eng.dma_start(out=ov[:, r0 : r0 + 4, :], in_=osb[:])

            for k in range(NT):
                r0 = 4 * k
                p1 = psM1.tile([128, 4, W], F32, tag="m1")
                p2 = psM2.tile([128, 4, W], F32, tag="m2")
                for t, (di, dj) in enumerate(taps):
                    nc.tensor.matmul(
                        p1[:],
                        lhsT=weff[t][0:64, :],
                        rhs=xpad[0:64, r0 + di : r0 + di + 4, dj : dj + W],
                        start=(t == 0),
                        stop=(t == 8),
                        tile_position=(0, 0),
                    )
                    nc.tensor.matmul(
                        p2[:],
                        lhsT=weff[t][64:128, :],
                        rhs=xpad[64:128, r0 + di : r0 + di + 4, dj : dj + W],
                        start=(t == 0),
                        stop=(t == 8),
                        tile_position=(64, 0),
                    )
                # Issue the lagged epilogue BEFORE this tile's relu evictions:
                # r1pad dependencies are tracked whole-tile, so conv2(k-2)
                # must not see evict(k) as its producer or PE stalls on ACT.
                if len(lag) == 2:
                    epilogue(*lag.pop(0))
                nc.scalar.activation(
                    out=r1pad[0:16, r0 + 1 : r0 + 5, 1 : W + 1],
                    in_=p1[0:16],
                    func=AF.Relu,
                )
                nc.scalar.activation(
                    out=r1pad[64:80, r0 + 1 : r0 + 5, 1 : W + 1],
                    in_=p2[64:80],
                    func=AF.Relu,
                )
                lag.append((k, p1, p2))
            while lag:
                epilogue(*lag.pop(0))

    nc.finalize()
    return nc


def _prep_params(weight, se_w1, se_w2, fc_w1, fc_w2):
    wt = weight.reshape(C, C, 9).transpose(1, 2, 0)  # [c, t, o]
    s1 = se_w1.reshape(16, C, 9).transpose(1, 2, 0)  # [c, t, s]
    wch = np.zeros((128, 9, 128), np.float32)
    wch[:64, :, 0:16] = s1  # chain-i0: relu cols
    wch[:64, :, 64:128] = wt  # chain-i0: main cols
    wch[64:, :, 0:64] = wt  # chain-i1: main cols
    wch[64:, :, 64:80] = s1  # chain-i1: relu cols
    se2 = np.zeros((128, 9, 128), np.float32)
    s2 = se_w2.reshape(16, 9)  # [s, t]
    se2[0:16, :, 64:128] = s2[:, :, None]  # relu1_i0 rows -> A_i0 @ 64-127
    se2[64:80, :, 0:64] = s2[:, :, None]  # relu1_i1 rows -> A_i1 @ 0-63
    fc1 = np.zeros((128, 4), np.float32)
    f1 = fc_w1.T.astype(np.float32) / float(H * W)  # fold 1/HW into fc1
    fc1[:64] = f1
    fc1[64:] = f1
    fc2 = fc_w2.T.astype(np.float32)  # [4, 576]
    return wch, se2, fc1, fc2


def kernel(x, weight, se_w1, se_w2, fc_w1, fc_w2):
    x = np.ascontiguousarray(x, np.float32)
    wch, se2, fc1, fc2 = _prep_params(
        np.asarray(weight, np.float32),
        np.asarray(se_w1, np.float32),
        np.asarray(se_w2, np.float32),
        np.asarray(fc_w1, np.float32),
        np.asarray(fc_w2, np.float32),
    )
    if "nc" not in _CACHED:
        _CACHED["nc"] = _build_nc()
    nc = _CACHED["nc"]
    # Shard slots are swapped: chain-i1 (xpad partitions 64-127, shard slot
    # 1) lands on dram out partitions 0-63 (slot 0), so feed (x[2i+1], x[2i])
    # and the outputs come back in natural order.
    in_maps = [
        {
            "x": np.ascontiguousarray(x[[BPC * i + 1, BPC * i]]),
            "wch": wch,
            "se2blk": se2,
            "fc1t": fc1,
            "fc2t": fc2,
        }
        for i in range(N_CORES)
    ]
    res = run_bass_kernel_spmd(
        nc, in_maps, core_ids=list(range(N_CORES)), trace=TRACE
    )
    if TRACE:
        print(f"HW exec time: {res.exec_time_ns} ns")
        print(f"mean exec time: {res.mean_exec_time_ns} ns")
        _CACHED["res"] = res
    out = np.concatenate([r["out"] for r in res.results], axis=0)
    return out.reshape(B, C, H, W).astype(np.float32)
